# revision 1
# baseline (speedup 1.0000x reference)
"""Trainium2 Bass kernel for the BGNN (3-layer GCN x 2 branches + mean-pool + MLP).

Contract: kernel(**inputs) takes FULL numpy inputs (keys as in
reference.setup_inputs()) and returns the FULL [G, 2] float32 output.
Internally: shards nodes+edges across 8 NeuronCores (dst-sharding),
gathers remote source features via dma_gather from a replicated
(AllGather'd) bf16 feature table, reduces messages with one-hot PE
matmuls, and evaluates the pooled MLP redundantly on every core.
"""
import sys

sys.path.insert(0, "/opt/trn_rl_repo")

import numpy as np
import ml_dtypes

import concourse.bacc as bacc
import concourse.bass as bass
import concourse.mybir as mybir
import concourse.tile as tile
from concourse.bass_utils import run_bass_kernel_spmd

P = 128
NCORE = 8
G = 64               # graphs per batch (fixed by the problem)
WG = 7               # windows per gather group
CHUNK_ROWS = 25088   # table rows addressable per int16 index chunk (<=32767)

last_results = None  # set by _run for test harness introspection


def _ceil_to(x, m):
    return (x + m - 1) // m * m


def _prep_branch(edge_index, batch, n, npad, sh, nw, ng, nchunk, cap=None):
    """Bucket edges by (dst-core, dst-window-group, src-chunk, window); build
    the padded int16 gather-index array, bf16 window-local dst array, dis,
    batch-local tiles and inverse counts."""
    src = edge_index[0].astype(np.int64)
    dst = edge_index[1].astype(np.int64)
    e = src.shape[0]

    deg = np.bincount(dst, minlength=n).astype(np.float32) + 1.0
    dis = deg ** -0.5                                   # [n]

    core = dst // sh
    win = (dst % sh) // P
    chunk = src // CHUNK_ROWS
    grp = win // WG
    win_in = win % WG
    key = ((core * ng + grp) * nchunk + chunk) * WG + win_in
    nbuckets = NCORE * ng * nchunk * WG
    order = np.argsort(key, kind="stable")
    key_s = key[order]
    counts = np.bincount(key_s, minlength=nbuckets)
    need = max(int(_ceil_to(counts.max(), P)), P)
    if cap is None:
        cap = need
    assert cap >= need
    bpb = cap // P

    starts = np.zeros(nbuckets, np.int64)
    np.cumsum(counts[:-1], out=starts[1:])
    rank = np.arange(e, dtype=np.int64) - starts[key_s]
    slot = key_s * cap + rank

    idx_flat = np.zeros(nbuckets * cap, np.int16)
    dl_flat = np.full(nbuckets * cap, -1.0, np.float32)
    idx_flat[slot] = (src[order] - chunk[order] * CHUNK_ROWS).astype(np.int16)
    dl_flat[slot] = (dst[order] % P).astype(np.float32)
    # pad idx slots repeat the bucket's last valid index (duplicate fetch, no effect)
    idx_mat = idx_flat.reshape(nbuckets, cap)
    has = counts > 0
    lastv = np.zeros(nbuckets, np.int16)
    lastv[has] = idx_mat[has, np.minimum(counts[has] - 1, cap - 1)]
    pad_mask = np.arange(cap)[None, :] >= counts[:, None]
    idx_mat[pad_mask] = np.broadcast_to(lastv[:, None], idx_mat.shape)[pad_mask]

    # per-core wrapped layouts
    idx_pc = idx_mat.reshape(NCORE, ng * nchunk * WG * cap)
    idx_w = idx_pc.reshape(NCORE, -1, 16).transpose(0, 2, 1)      # [NCORE,16,cols]
    idx_w = np.ascontiguousarray(np.tile(idx_w, (1, 8, 1)))       # [NCORE,128,cols]
    dl_pc = dl_flat.reshape(NCORE, ng * nchunk * WG * bpb, P)
    dl_w = np.ascontiguousarray(dl_pc.transpose(0, 2, 1)).astype(ml_dtypes.bfloat16)

    dis_pad = np.ones(npad, np.float32)
    dis_pad[:n] = dis
    dis_t = np.ascontiguousarray(dis_pad.reshape(NCORE, nw, P).transpose(0, 2, 1))
    bl_pad = np.full(npad, -1.0, np.float32)
    bl_pad[:n] = batch.astype(np.float32)
    bl_t = np.ascontiguousarray(
        bl_pad.reshape(NCORE, nw, P).transpose(0, 2, 1)).astype(ml_dtypes.bfloat16)

    cnt = np.bincount(batch.astype(np.int64), minlength=G).astype(np.float32)
    inv_cnt = (1.0 / np.maximum(cnt, 1.0)).reshape(G, 1)

    return dict(idx=idx_w, dl=dl_w, dis=dis_t, bl=bl_t, inv_cnt=inv_cnt,
                cap=cap, need=need, bpb=bpb)


def _build_program(npad, sh, nw, ng, nchunk, bpb, d_feat, msg_bufs=2):
    nc = bacc.Bacc()
    bf16 = mybir.dt.bfloat16
    f32 = mybir.dt.float32
    i16 = mybir.dt.int16
    cap = bpb * P
    nblk = ng * nchunk * WG * bpb
    idx_cols = ng * nchunk * WG * cap // 16
    gcols = WG * cap // 16
    gblk = WG * bpb
    DIMS = [d_feat, 32, 16, 8]

    # ---------------- parameters ----------------
    prm = {}
    for b in (0, 1):
        prm[f"x{b}"] = nc.declare_dram_parameter(f"x{b}", [sh, d_feat], f32, isOutput=False)
        prm[f"idx{b}"] = nc.declare_dram_parameter(f"idx{b}", [P, idx_cols], i16, isOutput=False)
        prm[f"dl{b}"] = nc.declare_dram_parameter(f"dl{b}", [P, nblk], bf16, isOutput=False)
        prm[f"dis{b}"] = nc.declare_dram_parameter(f"dis{b}", [P, nw], f32, isOutput=False)
        prm[f"bl{b}"] = nc.declare_dram_parameter(f"bl{b}", [P, nw], bf16, isOutput=False)
        prm[f"ic{b}"] = nc.declare_dram_parameter(f"ic{b}", [G, 1], f32, isOutput=False)
    ident_in = nc.declare_dram_parameter("ident", [P, P], f32, isOutput=False)
    iota128_in = nc.declare_dram_parameter("iota128", [P, P], bf16, isOutput=False)
    iota64_in = nc.declare_dram_parameter("iota64", [P, G], bf16, isOutput=False)
    W_in = [nc.declare_dram_parameter(f"W{l+1}", [DIMS[l], DIMS[l+1]], f32, isOutput=False) for l in range(3)]
    B_in = [nc.declare_dram_parameter(f"b{l+1}r", [P, DIMS[l+1]], f32, isOutput=False) for l in range(3)]
    mW1_in = nc.declare_dram_parameter("mW1", [16, 8], f32, isOutput=False)
    mb1_in = nc.declare_dram_parameter("mb1r", [G, 8], f32, isOutput=False)
    mW2_in = nc.declare_dram_parameter("mW2", [8, 2], f32, isOutput=False)
    mb2_in = nc.declare_dram_parameter("mb2r", [G, 2], f32, isOutput=False)
    out_p = nc.declare_dram_parameter("out", [G, 2], f32, isOutput=True)

    # ---------------- internal DRAM ----------------
    tabfull = nc.dram_tensor("tabfull", [npad, P], bf16)
    agin = [nc.dram_tensor(f"agin{l}", [sh, DIMS[l + 1]], bf16) for l in range(3)]
    agfull = [nc.dram_tensor(f"agfull{l}", [npad, DIMS[l + 1]], bf16) for l in range(3)]
    pool_in = nc.dram_tensor("pool_in", [G, 8], f32)
    pool_out = nc.dram_tensor("pool_out", [G, 8], f32)

    with tile.TileContext(nc) as tc:
        with (
            tc.tile_pool(name="const", bufs=1) as cp,
            tc.tile_pool(name="resident", bufs=1) as rp,
            tc.tile_pool(name="stream", bufs=msg_bufs) as spool,
            tc.tile_pool(name="small", bufs=3) as sm,
        ):
            # ---- constants ----
            ident = cp.tile([P, P], f32)
            nc.sync.dma_start(out=ident[:, :], in_=ident_in[:, :])
            iota128 = cp.tile([P, P], bf16)
            nc.sync.dma_start(out=iota128[:, :], in_=iota128_in[:, :])
            iota64 = cp.tile([P, G], bf16)
            nc.sync.dma_start(out=iota64[:, :], in_=iota64_in[:, :])
            Ws, Bs = [], []
            for l in range(3):
                w = cp.tile([DIMS[l], DIMS[l + 1]], f32, tag=f"w{l}")
                nc.sync.dma_start(out=w[:, :], in_=W_in[l][:, :])
                Ws.append(w)
                bb = cp.tile([P, DIMS[l + 1]], f32, tag=f"b{l}")
                nc.sync.dma_start(out=bb[:, :], in_=B_in[l][:, :])
                Bs.append(bb)
            mW1 = cp.tile([16, 8], f32)
            nc.sync.dma_start(out=mW1[:, :], in_=mW1_in[:, :])
            mb1 = cp.tile([G, 8], f32)
            nc.sync.dma_start(out=mb1[:, :], in_=mb1_in[:, :])
            mW2 = cp.tile([8, 2], f32)
            nc.sync.dma_start(out=mW2[:, :], in_=mW2_in[:, :])
            mb2 = cp.tile([G, 2], f32)
            nc.sync.dma_start(out=mb2[:, :], in_=mb2_in[:, :])

            pooled_cat = rp.tile([G, 16], f32)

            for br in (0, 1):
                dl_t = rp.tile([P, nblk], bf16, tag="dl")
                nc.sync.dma_start(out=dl_t[:, :], in_=prm[f"dl{br}"][:, :])
                dis_t = rp.tile([P, nw], f32, tag="dis")
                nc.sync.dma_start(out=dis_t[:, :], in_=prm[f"dis{br}"][:, :])
                bl_t = rp.tile([P, nw], bf16, tag="bl")
                nc.sync.dma_start(out=bl_t[:, :], in_=prm[f"bl{br}"][:, :])

                hself = rp.tile([P, nw, 32], f32, tag="hself")
                act = rp.tile([P, nw, 32], f32, tag="act")

                for l in range(3):
                    din, dout = DIMS[l], DIMS[l + 1]
                    # ======== table build: tab = dis * (act_in @ W) ========
                    pt_ctx = tc.tile_pool(name=f"pt_{br}_{l}", bufs=2, space="PSUM")
                    pt = pt_ctx.__enter__()
                    for t in range(nw):
                        if l == 0:
                            a_t = sm.tile([P, d_feat], f32, tag="a_in")
                            nc.sync.dma_start(out=a_t[:, :],
                                              in_=prm[f"x{br}"][t * P:(t + 1) * P, :])
                            a_ap = a_t[:, :]
                        else:
                            a_ap = act[:, t, :din]
                        aT_p = pt.tile([din, P], f32, tag="aT_p", space="PSUM")
                        nc.tensor.transpose(out=aT_p[:, :], in_=a_ap, identity=ident[:, :])
                        aT_s = sm.tile([din, P], f32, tag="aT_s")
                        nc.vector.tensor_copy(out=aT_s[:, :], in_=aT_p[:, :])
                        h_p = pt.tile([P, dout], f32, tag="h_p", space="PSUM")
                        nc.tensor.matmul(out=h_p[:, :], lhsT=aT_s[:, :], rhs=Ws[l][:, :],
                                         start=True, stop=True)
                        nc.vector.tensor_scalar_mul(out=hself[:, t, :dout], in0=h_p[:, :],
                                                    scalar1=dis_t[:, t:t + 1])
                        stage = sm.tile([P, dout], bf16, tag="stage")
                        nc.vector.tensor_copy(out=stage[:, :], in_=hself[:, t, :dout])
                        nc.sync.dma_start(out=agin[l][t * P:(t + 1) * P, :], in_=stage[:, :])
                    pt_ctx.__exit__(None, None, None)

                    # ======== AllGather table + expand into tabfull ========
                    nc.gpsimd.collective_compute(
                        "AllGather", mybir.AluOpType.bypass,
                        replica_groups=[list(range(NCORE))],
                        ins=[agin[l][:, :]], outs=[agfull[l][:, :]],
                    )
                    half = npad // 2
                    nc.sync.dma_start(out=tabfull[0:half, 0:dout],
                                      in_=agfull[l][0:half, :])
                    nc.sync.dma_start(out=tabfull[half:npad, 0:dout],
                                      in_=agfull[l][half:npad, :])

                    # ======== gather + one-hot reduce ========
                    pg_ctx = tc.tile_pool(name=f"pg_{br}_{l}", bufs=1, space="PSUM")
                    pg = pg_ctx.__enter__()
                    for g in range(ng):
                        aggs = [pg.tile([P, 32], f32, tag=f"agg{wi}",
                                        name=f"agg_{br}_{l}_{g}_{wi}", space="PSUM")
                                for wi in range(WG)]
                        for c in range(nchunk):
                            gi = g * nchunk + c
                            idx_t = spool.tile([P, gcols], i16, tag="idx")
                            nc.sync.dma_start(out=idx_t[:, :],
                                              in_=prm[f"idx{br}"][:, gi * gcols:(gi + 1) * gcols])
                            msgs = spool.tile([P, gblk, P], bf16, tag="msgs")
                            nc.gpsimd.dma_gather(
                                out_ap=msgs[:, :, :],
                                in_ap=tabfull[c * CHUNK_ROWS:min((c + 1) * CHUNK_ROWS, npad), :],
                                idxs_ap=idx_t[:, :], num_idxs=WG * cap,
                                num_idxs_reg=WG * cap, elem_size=P,
                                single_packet=False,
                            )
                            sd = spool.tile([P, gblk, P], bf16, tag="sd")
                            nc.vector.tensor_tensor(
                                out=sd[:, :, :],
                                in0=dl_t[:, gi * gblk:(gi + 1) * gblk, None].to_broadcast([P, gblk, P]),
                                in1=iota128[:, None, :].to_broadcast([P, gblk, P]),
                                op=mybir.AluOpType.is_equal,
                            )
                            for wi in range(WG):
                                for b2 in range(bpb):
                                    blk = wi * bpb + b2
                                    nc.tensor.matmul(
                                        out=aggs[wi][:, :dout],
                                        lhsT=sd[:, blk, :], rhs=msgs[:, blk, :dout],
                                        start=(c == 0 and b2 == 0),
                                        stop=(c == nchunk - 1 and b2 == bpb - 1),
                                    )
                        for wi in range(WG):
                            w = g * WG + wi
                            t1 = sm.tile([P, dout], f32, tag="post1")
                            nc.vector.tensor_add(out=t1[:, :], in0=aggs[wi][:, :dout],
                                                 in1=hself[:, w, :dout])
                            nc.vector.tensor_scalar(
                                out=act[:, w, :dout], in0=t1[:, :],
                                scalar1=dis_t[:, w:w + 1], scalar2=None,
                                op0=mybir.AluOpType.mult,
                            )
                            nc.vector.tensor_add(out=act[:, w, :dout],
                                                 in0=act[:, w, :dout], in1=Bs[l][:, :])
                            if l < 2:
                                nc.vector.tensor_scalar_max(out=act[:, w, :dout],
                                                            in0=act[:, w, :dout],
                                                            scalar1=0.0)
                    pg_ctx.__exit__(None, None, None)

                # ======== mean-pool branch -> pooled_cat[:, br*8 : br*8+8] ========
                pp_ctx = tc.tile_pool(name=f"pp_{br}", bufs=1, space="PSUM")
                pp = pp_ctx.__enter__()
                pool_p = pp.tile([G, 8], f32, tag="h_p", name="pool_p", space="PSUM")
                for t in range(nw):
                    oh = sm.tile([P, G], bf16, tag="pool_oh")
                    nc.vector.tensor_tensor(
                        out=oh[:, :],
                        in0=bl_t[:, t:t + 1].to_broadcast([P, G]),
                        in1=iota64[:, :], op=mybir.AluOpType.is_equal,
                    )
                    a_bf = sm.tile([P, 8], bf16, tag="pool_in_bf")
                    nc.vector.tensor_copy(out=a_bf[:, :], in_=act[:, t, :8])
                    nc.tensor.matmul(out=pool_p[:, :], lhsT=oh[:, :], rhs=a_bf[:, :],
                                     start=(t == 0), stop=(t == nw - 1))
                pool_s = sm.tile([G, 8], f32, tag="pool_s")
                nc.vector.tensor_copy(out=pool_s[:, :], in_=pool_p[:, :])
                nc.sync.dma_start(out=pool_in[:, :], in_=pool_s[:, :])
                nc.gpsimd.collective_compute(
                    "AllReduce", mybir.AluOpType.add,
                    replica_groups=[list(range(NCORE))],
                    ins=[pool_in[:, :]], outs=[pool_out[:, :]],
                )
                pool_r = sm.tile([G, 8], f32, tag="pool_r")
                nc.sync.dma_start(out=pool_r[:, :], in_=pool_out[:, :])
                ic_t = sm.tile([G, 1], f32, tag="ic")
                nc.sync.dma_start(out=ic_t[:, :], in_=prm[f"ic{br}"][:, :])
                nc.vector.tensor_scalar_mul(out=pooled_cat[:, br * 8:br * 8 + 8],
                                            in0=pool_r[:, :], scalar1=ic_t[:, :])
                pp_ctx.__exit__(None, None, None)

            # ======== MLP: relu(cat @ mW1 + mb1) @ mW2 + mb2 ========
            pm_ctx = tc.tile_pool(name="pm", bufs=1, space="PSUM")
            pm = pm_ctx.__enter__()
            pcT_p = pm.tile([16, G], f32, tag="aT_p", name="pcT_p", space="PSUM")
            nc.tensor.transpose(out=pcT_p[:, :], in_=pooled_cat[:, :], identity=ident[:G, :G])
            pcT_s = sm.tile([16, G], f32, tag="pcT_s")
            nc.vector.tensor_copy(out=pcT_s[:, :], in_=pcT_p[:, :])
            m1_p = pm.tile([G, 8], f32, tag="h_p", name="m1_p", space="PSUM")
            nc.tensor.matmul(out=m1_p[:, :], lhsT=pcT_s[:, :], rhs=mW1[:, :],
                             start=True, stop=True)
            m1_s = sm.tile([G, 8], f32, tag="m1s")
            nc.vector.tensor_add(out=m1_s[:, :], in0=m1_p[:, :], in1=mb1[:, :])
            nc.vector.tensor_scalar_max(out=m1_s[:, :], in0=m1_s[:, :], scalar1=0.0)
            m1T_p = pm.tile([8, G], f32, tag="aT_p2", name="m1T_p", space="PSUM")
            nc.tensor.transpose(out=m1T_p[:, :], in_=m1_s[:, :], identity=ident[:G, :G])
            m1T_s = sm.tile([8, G], f32, tag="m1Ts")
            nc.vector.tensor_copy(out=m1T_s[:, :], in_=m1T_p[:, :])
            m2_p = pm.tile([G, 2], f32, tag="h_p2", name="m2_p", space="PSUM")
            nc.tensor.matmul(out=m2_p[:, :], lhsT=m1T_s[:, :], rhs=mW2[:, :],
                             start=True, stop=True)
            m2_s = sm.tile([G, 2], f32, tag="m2s")
            nc.vector.tensor_add(out=m2_s[:, :], in0=m2_p[:, :], in1=mb2[:, :])
            nc.sync.dma_start(out=out_p[:, :], in_=m2_s[:, :])
            pm_ctx.__exit__(None, None, None)

    nc.compile()
    return nc


def _run(inputs, trace=False, msg_bufs=2):
    global last_results
    x0 = np.asarray(inputs["x0"], np.float32)
    x1 = np.asarray(inputs["x1"], np.float32)
    n, d_feat = x0.shape
    ei0 = np.asarray(inputs["edge_index0"])
    ei1 = np.asarray(inputs["edge_index1"])
    b0 = np.asarray(inputs["batch0"])
    b1 = np.asarray(inputs["batch1"])

    per_core = (n + NCORE - 1) // NCORE
    sh = _ceil_to(per_core, P * WG)       # windows per core divisible by WG
    npad = sh * NCORE
    nw = sh // P
    ng = nw // WG
    nchunk = max(1, (npad + CHUNK_ROWS - 1) // CHUNK_ROWS)

    pb0 = _prep_branch(ei0, b0, n, npad, sh, nw, ng, nchunk)
    pb1 = _prep_branch(ei1, b1, n, npad, sh, nw, ng, nchunk)
    cap = max(pb0["cap"], pb1["cap"])
    if pb0["cap"] != cap:
        pb0 = _prep_branch(ei0, b0, n, npad, sh, nw, ng, nchunk, cap=cap)
    if pb1["cap"] != cap:
        pb1 = _prep_branch(ei1, b1, n, npad, sh, nw, ng, nchunk, cap=cap)
    bpb = cap // P

    xp = []
    for x in (x0, x1):
        t = np.zeros((npad, d_feat), np.float32)
        t[:n] = x
        xp.append(t.reshape(NCORE, sh, d_feat))

    ident = np.eye(P, dtype=np.float32)
    iota128 = np.broadcast_to(np.arange(P, dtype=np.float32), (P, P)).astype(ml_dtypes.bfloat16)
    iota64 = np.broadcast_to(np.arange(G, dtype=np.float32), (P, G)).astype(ml_dtypes.bfloat16)

    def wgt(name):
        return np.asarray(inputs[name], np.float32)

    common = dict(
        ident=ident, iota128=np.ascontiguousarray(iota128),
        iota64=np.ascontiguousarray(iota64),
        W1=wgt("W1"), W2=wgt("W2"), W3=wgt("W3"),
        b1r=np.broadcast_to(wgt("b1"), (P, 32)).copy(),
        b2r=np.broadcast_to(wgt("b2"), (P, 16)).copy(),
        b3r=np.broadcast_to(wgt("b3"), (P, 8)).copy(),
        mW1=wgt("mW1"), mb1r=np.broadcast_to(wgt("mb1"), (G, 8)).copy(),
        mW2=wgt("mW2"), mb2r=np.broadcast_to(wgt("mb2"), (G, 2)).copy(),
        ic0=pb0["inv_cnt"], ic1=pb1["inv_cnt"],
    )
    in_maps = []
    for c in range(NCORE):
        m = dict(common)
        m["x0"] = np.ascontiguousarray(xp[0][c])
        m["x1"] = np.ascontiguousarray(xp[1][c])
        for name, pb in (("0", pb0), ("1", pb1)):
            m[f"idx{name}"] = pb["idx"][c]
            m[f"dl{name}"] = pb["dl"][c]
            m[f"dis{name}"] = pb["dis"][c]
            m[f"bl{name}"] = pb["bl"][c]
        in_maps.append(m)

    nc = _build_program(npad, sh, nw, ng, nchunk, bpb, d_feat, msg_bufs=msg_bufs)
    res = run_bass_kernel_spmd(nc, in_maps, list(range(NCORE)), trace=trace)
    last_results = res
    return np.asarray(res.results[0]["out"], np.float32)


def kernel(**inputs):
    return _run(inputs, trace=False)



# revision 13
# speedup vs baseline: 3.9479x; 3.9479x over previous
"""Trainium2 Bass kernel for the BGNN (3-layer GCN x 2 branches + mean-pool + MLP).

Contract: kernel(**inputs) takes FULL numpy inputs (keys as in
reference.setup_inputs()) and returns the FULL [G, 2] float32 output.

Design (v2):
- dst-shards nodes+edges across 8 NeuronCores; self-loops appended as real
  edges (norm dis_src*dis_dst covers the self term exactly).
- Deferred weights: gathers move PRE-W activations (aggregation commutes with
  the linear x@W).  Layer-0's gather table is dis*x, built on host -- no
  layer-0 AllGather or table build.  W3/b3/mean-pool all commute past the
  last aggregation, so layer 2 needs no per-window weight matmul at all.
- dma_gather descriptor generation is spread over all 4 SWDGE queues
  (= 4 Q7 core pairs) round-robin; ~4x the single-queue throughput.
- One-hot reduce on the tensor engine (lhsT = dst-slot one-hot, rhs = msgs).
- Per-window post ops avoid TENSOR_SCALAR (pathologically slow on this DVE
  build): everything is tensor_tensor with broadcast APs.
- Branches interleaved per layer so each branch's AllGather overlaps the
  other branch's gather/compute; one combined [G,32] AllReduce for the pools.
"""
import sys

sys.path.insert(0, "/opt/trn_rl_repo")

import numpy as np
import ml_dtypes

import concourse.bacc as bacc
import concourse.mybir as mybir
import concourse.tile as tile
from concourse.bass_utils import run_bass_kernel_spmd

P = 128
NCORE = 8
G = 64               # graphs per batch
WG = 2               # windows per gather group (2 PSUM agg banks x 2 bufs)
CHUNK_ROWS = 25088   # table rows per int16-indexed chunk (npad = 4*25088)
NQ = 4               # SWDGE queues (Q7 core pairs)

last_results = None  # set by _run for test harness introspection


def _ceil_to(x, m):
    return (x + m - 1) // m * m


def _prep_branch(edge_index, batch, n, npad, sh, nw, ng, nchunk, cap=None):
    """Append self-loops, bucket edges by (dst-core, group, src-chunk, window),
    build the padded int16 gather-index array, per-block dst-local array,
    dis / pool-one-hot tiles and inverse counts."""
    src0 = edge_index[0].astype(np.int64)
    dst0 = edge_index[1].astype(np.int64)
    self_ix = np.arange(n, dtype=np.int64)
    src = np.concatenate([src0, self_ix])
    dst = np.concatenate([dst0, self_ix])
    e = src.shape[0]

    deg = np.bincount(dst, minlength=n).astype(np.float32)  # includes self
    dis = deg ** -0.5                                       # [n]

    core = dst // sh
    win = (dst % sh) // P
    chunk = src // CHUNK_ROWS
    grp = win // WG
    win_in = win % WG
    key = ((core * ng + grp) * nchunk + chunk) * WG + win_in
    nbuckets = NCORE * ng * nchunk * WG
    order = np.argsort(key, kind="stable")
    key_s = key[order]
    counts = np.bincount(key_s, minlength=nbuckets)
    need = max(int(_ceil_to(counts.max(), P)), P)
    if cap is None:
        cap = need
    assert cap >= need
    bpb = cap // P

    starts = np.zeros(nbuckets, np.int64)
    np.cumsum(counts[:-1], out=starts[1:])
    rank = np.arange(e, dtype=np.int64) - starts[key_s]
    slot = key_s * cap + rank

    idx_flat = np.zeros(nbuckets * cap, np.int16)
    dl_flat = np.full(nbuckets * cap, -1.0, np.float32)
    idx_flat[slot] = (src[order] - chunk[order] * CHUNK_ROWS).astype(np.int16)
    dl_flat[slot] = (dst[order] % P).astype(np.float32)
    # pad idx slots repeat the bucket's last valid index (harmless duplicate fetch)
    idx_mat = idx_flat.reshape(nbuckets, cap)
    has = counts > 0
    lastv = np.zeros(nbuckets, np.int16)
    lastv[has] = idx_mat[has, np.minimum(counts[has] - 1, cap - 1)]
    pad_mask = np.arange(cap)[None, :] >= counts[:, None]
    idx_mat[pad_mask] = np.broadcast_to(lastv[:, None], idx_mat.shape)[pad_mask]

    # per-core wrapped layouts.  slot order within a core is
    # (g, chunk, win_in, block) which matches the device loop exactly.
    idx_pc = idx_mat.reshape(NCORE, ng * nchunk * WG * cap)
    idx_w = idx_pc.reshape(NCORE, -1, 16).transpose(0, 2, 1)      # [NCORE,16,cols]
    idx_w = np.ascontiguousarray(np.tile(idx_w, (1, 8, 1)))       # [NCORE,128,cols]
    dl_pc = dl_flat.reshape(NCORE, ng * nchunk * WG * bpb, P)
    dl_w = np.ascontiguousarray(dl_pc.transpose(0, 2, 1)).astype(ml_dtypes.bfloat16)

    dis_pad = np.ones(npad, np.float32)
    dis_pad[:n] = dis
    dis_t = np.ascontiguousarray(dis_pad.reshape(NCORE, nw, P).transpose(0, 2, 1))

    # pool one-hot oh[p, t, g] = (batch[node] == g); 0 for padding nodes
    bl_pad = np.full(npad, -1, np.int64)
    bl_pad[:n] = batch.astype(np.int64)
    bl_c = bl_pad.reshape(NCORE, nw, P)
    oh = (bl_c[:, :, :, None] == np.arange(G)[None, None, None, :])
    oh_t = np.ascontiguousarray(
        oh.transpose(0, 2, 1, 3)).astype(ml_dtypes.bfloat16)      # [NCORE,P,nw,G]

    cnt = np.bincount(batch.astype(np.int64), minlength=G).astype(np.float32)
    inv_cnt = (1.0 / np.maximum(cnt, 1.0)).reshape(G, 1)

    return dict(idx=idx_w, dl=dl_w, dis=dis_t, oh=oh_t, inv_cnt=inv_cnt,
                dis_full=dis_pad, cap=cap, need=need, bpb=bpb)


def _build_program(npad, sh, nw, ng, nchunk, bpb, d_feat):
    nc = bacc.Bacc(num_swdge_queues=NQ)
    bf16 = mybir.dt.bfloat16
    f32 = mybir.dt.float32
    i16 = mybir.dt.int16
    cap = bpb * P
    nblk_gc = WG * bpb                 # blocks per (group, chunk)
    nblk = ng * nchunk * nblk_gc       # blocks per core per branch
    idx_cols = nblk * 8                # int16 idx columns ([16]-wrapped)
    DIMS = [d_feat, 32, 16, 8]         # feature widths; gathers move DIMS[l]

    # ---------------- parameters ----------------
    prm = {}
    for b in (0, 1):
        prm[f"tab{b}"] = nc.declare_dram_parameter(f"tab{b}", [npad, P], bf16, isOutput=False)
        prm[f"idx{b}"] = nc.declare_dram_parameter(f"idx{b}", [P, idx_cols], i16, isOutput=False)
        prm[f"dl{b}"] = nc.declare_dram_parameter(f"dl{b}", [P, nblk], bf16, isOutput=False)
        prm[f"dis{b}"] = nc.declare_dram_parameter(f"dis{b}", [P, nw], f32, isOutput=False)
        prm[f"oh{b}"] = nc.declare_dram_parameter(f"oh{b}", [P, nw * G], bf16, isOutput=False)
        prm[f"ic{b}"] = nc.declare_dram_parameter(f"ic{b}", [G, 1], f32, isOutput=False)
    ident_in = nc.declare_dram_parameter("ident", [P, P], f32, isOutput=False)
    identb_in = nc.declare_dram_parameter("identb", [P, P], bf16, isOutput=False)
    iota128_in = nc.declare_dram_parameter("iota128", [P, P], bf16, isOutput=False)
    W_in = [nc.declare_dram_parameter(f"W{l+1}", [DIMS[l], DIMS[l+1]], bf16, isOutput=False) for l in range(2)]
    W3_in = nc.declare_dram_parameter("W3", [16, 8], f32, isOutput=False)
    B_in = [nc.declare_dram_parameter(f"b{l+1}r", [P, DIMS[l+1]], f32, isOutput=False) for l in range(2)]
    b3_in = nc.declare_dram_parameter("b3r", [G, 8], f32, isOutput=False)
    mW1_in = nc.declare_dram_parameter("mW1", [16, 8], f32, isOutput=False)
    mb1_in = nc.declare_dram_parameter("mb1r", [G, 8], f32, isOutput=False)
    mW2_in = nc.declare_dram_parameter("mW2", [8, 2], f32, isOutput=False)
    mb2_in = nc.declare_dram_parameter("mb2r", [G, 2], f32, isOutput=False)
    out_p = nc.declare_dram_parameter("out", [G, 2], f32, isOutput=True)

    # ---------------- internal DRAM ----------------
    tabfull = [nc.dram_tensor(f"tabfull{b}", [npad, P], bf16) for b in (0, 1)]
    agin = {(b, l): nc.dram_tensor(f"agin{b}_{l}", [sh, DIMS[l + 1]], bf16)
            for b in (0, 1) for l in (0, 1)}
    agfull = {(b, l): nc.dram_tensor(f"agfull{b}_{l}", [npad, DIMS[l + 1]], bf16,
                                     addr_space="Shared")
              for b in (0, 1) for l in (0, 1)}
    pool_in = nc.dram_tensor("pool_in", [G, 32], f32)
    pool_out = nc.dram_tensor("pool_out", [G, 32], f32, addr_space="Shared")

    with tile.TileContext(nc) as tc:
        with (
            tc.tile_pool(name="const", bufs=1) as cp,
            tc.tile_pool(name="resident", bufs=1) as rp,
            tc.tile_pool(name="gmsg", bufs=2) as gm,
            tc.tile_pool(name="gsd", bufs=1) as gs,
            tc.tile_pool(name="small", bufs=3) as sm,
        ):
            # ---- constants ----
            ident = cp.tile([P, P], f32)
            nc.sync.dma_start(out=ident[:, :], in_=ident_in[:, :])
            identb = cp.tile([P, P], bf16)
            nc.sync.dma_start(out=identb[:, :], in_=identb_in[:, :])
            iota128 = cp.tile([P, P], bf16)
            nc.sync.dma_start(out=iota128[:, :], in_=iota128_in[:, :])
            zcol = cp.tile([P, 1], f32)
            nc.vector.memset(zcol[:, :], 0.0)
            Ws, Bs = [], []
            for l in range(2):
                w = cp.tile([DIMS[l], DIMS[l + 1]], bf16, tag=f"w{l}")
                nc.sync.dma_start(out=w[:, :], in_=W_in[l][:, :])
                Ws.append(w)
                bb = cp.tile([P, DIMS[l + 1]], f32, tag=f"b{l}")
                nc.sync.dma_start(out=bb[:, :], in_=B_in[l][:, :])
                Bs.append(bb)
            W3 = cp.tile([16, 8], f32)
            nc.sync.dma_start(out=W3[:, :], in_=W3_in[:, :])
            b3 = cp.tile([G, 8], f32)
            nc.sync.dma_start(out=b3[:, :], in_=b3_in[:, :])
            mW1 = cp.tile([16, 8], f32)
            nc.sync.dma_start(out=mW1[:, :], in_=mW1_in[:, :])
            mb1 = cp.tile([G, 8], f32)
            nc.sync.dma_start(out=mb1[:, :], in_=mb1_in[:, :])
            mW2 = cp.tile([8, 2], f32)
            nc.sync.dma_start(out=mW2[:, :], in_=mW2_in[:, :])
            mb2 = cp.tile([G, 2], f32)
            nc.sync.dma_start(out=mb2[:, :], in_=mb2_in[:, :])

            dl_t, dis_t, oh_t, ic_t = {}, {}, {}, {}
            for b in (0, 1):
                dl_t[b] = rp.tile([P, nblk], bf16, tag=f"dl{b}", name=f"dl_t{b}")
                nc.sync.dma_start(out=dl_t[b][:, :], in_=prm[f"dl{b}"][:, :])
                dis_t[b] = rp.tile([P, nw], f32, tag=f"dis{b}", name=f"dis_t{b}")
                nc.sync.dma_start(out=dis_t[b][:, :], in_=prm[f"dis{b}"][:, :])
                oh_t[b] = rp.tile([P, nw * G], bf16, tag=f"oh{b}", name=f"oh_t{b}")
                nc.sync.dma_start(out=oh_t[b][:, :], in_=prm[f"oh{b}"][:, :])
                ic_t[b] = rp.tile([G, 1], f32, tag=f"ic{b}", name=f"ic_t{b}")
                nc.sync.dma_start(out=ic_t[b][:, :], in_=prm[f"ic{b}"][:, :])

            pooled_cat = rp.tile([G, 16], f32)
            qctr = 0

            for l in range(3):
                din = DIMS[l]
                dout = DIMS[l + 1] if l < 2 else None
                for br in (0, 1):
                    tabsrc = prm[f"tab{br}"] if l == 0 else tabfull[br]
                    pg_ctx = tc.tile_pool(name=f"pg_{l}_{br}", bufs=2, space="PSUM")
                    pg = pg_ctx.__enter__()
                    pt_ctx = tc.tile_pool(name=f"pt_{l}_{br}", bufs=1, space="PSUM")
                    pt = pt_ctx.__enter__()
                    if l == 2:
                        pp_ctx = tc.tile_pool(name=f"pp_{br}", bufs=1, space="PSUM")
                        pp = pp_ctx.__enter__()
                        pool_p = pp.tile([G, 16], f32, tag="poolp",
                                         name=f"pool_p{br}", space="PSUM")
                    for g in range(ng):
                        aggs = [pg.tile([P, din], f32, tag=f"agg{wi}",
                                        name=f"agg_{l}_{br}_{g}_{wi}", space="PSUM")
                                for wi in range(WG)]
                        for c in range(nchunk):
                            gc_base = (g * nchunk + c) * nblk_gc
                            q = qctr % NQ
                            qctr += 1
                            idx_t = gm.tile([P, nblk_gc * 8], i16, tag=f"idx{q}")
                            col0 = gc_base * 8
                            nc.sync.dma_start(
                                out=idx_t[:, :],
                                in_=prm[f"idx{br}"][:, col0:col0 + nblk_gc * 8])
                            msgs = gm.tile([P, nblk_gc, P], bf16, tag=f"msgs{q}")
                            nc.gpsimd.dma_gather(
                                out_ap=msgs[:, :, :],
                                in_ap=tabsrc[c * CHUNK_ROWS:(c + 1) * CHUNK_ROWS, :],
                                idxs_ap=idx_t[:, :], num_idxs=nblk_gc * P,
                                num_idxs_reg=nblk_gc * P, elem_size=P,
                                single_packet=False,
                                queue_num=q,
                            )
                            sd = gs.tile([P, nblk_gc, P], bf16, tag=f"sd{q}")
                            nc.vector.tensor_tensor(
                                out=sd[:, :, :],
                                in0=dl_t[br][:, gc_base:gc_base + nblk_gc, None]
                                    .to_broadcast([P, nblk_gc, P]),
                                in1=iota128[:, None, :].to_broadcast([P, nblk_gc, P]),
                                op=mybir.AluOpType.is_equal,
                            )
                            for j in range(nblk_gc):
                                wi = j // bpb
                                b2 = j % bpb
                                nc.tensor.matmul(
                                    out=aggs[wi][:, :din],
                                    lhsT=sd[:, j, :], rhs=msgs[:, j, :din],
                                    start=(c == 0 and b2 == 0),
                                    stop=(c == nchunk - 1 and b2 == bpb - 1),
                                )
                        # ---- per-window post-aggregation ----
                        for wi in range(WG):
                            w = g * WG + wi
                            if l < 2:
                                # t = (sum msgs) * dis  -> [P, din] bf16
                                t_s = sm.tile([P, din], bf16, tag="ts")
                                nc.vector.tensor_tensor(
                                    out=t_s[:, :], in0=aggs[wi][:, :din],
                                    in1=dis_t[br][:, w:w + 1].to_broadcast([P, din]),
                                    op=mybir.AluOpType.mult)
                                tT_p = pt.tile([din, P], bf16, tag="tT", space="PSUM")
                                nc.tensor.transpose(out=tT_p[:, :], in_=t_s[:, :],
                                                    identity=identb[:, :])
                                tT_s = sm.tile([din, P], bf16, tag="tTs")
                                nc.vector.tensor_copy(out=tT_s[:, :], in_=tT_p[:, :])
                                h_p = pt.tile([P, dout], f32, tag="hp", space="PSUM")
                                nc.tensor.matmul(out=h_p[:, :], lhsT=tT_s[:, :],
                                                 rhs=Ws[l][:, :], start=True, stop=True)
                                u_s = sm.tile([P, dout], f32, tag="us")
                                nc.vector.tensor_tensor(
                                    out=u_s[:, :], in0=h_p[:, :], in1=Bs[l][:, :],
                                    op=mybir.AluOpType.add)
                                v_s = sm.tile([P, dout], f32, tag="vs")
                                nc.vector.tensor_tensor(
                                    out=v_s[:, :], in0=u_s[:, :],
                                    in1=zcol[:, 0:1].to_broadcast([P, dout]),
                                    op=mybir.AluOpType.max)
                                stage = sm.tile([P, dout], bf16, tag="stage")
                                nc.vector.tensor_tensor(
                                    out=stage[:, :], in0=v_s[:, :],
                                    in1=dis_t[br][:, w:w + 1].to_broadcast([P, dout]),
                                    op=mybir.AluOpType.mult)
                                nc.sync.dma_start(
                                    out=agin[(br, l)][w * P:(w + 1) * P, :],
                                    in_=stage[:, :])
                            else:
                                s3 = sm.tile([P, 16], bf16, tag="s3")
                                nc.vector.tensor_tensor(
                                    out=s3[:, :], in0=aggs[wi][:, :16],
                                    in1=dis_t[br][:, w:w + 1].to_broadcast([P, 16]),
                                    op=mybir.AluOpType.mult)
                                nc.tensor.matmul(
                                    out=pool_p[:, :],
                                    lhsT=oh_t[br][:, w * G:(w + 1) * G], rhs=s3[:, :],
                                    start=(w == 0), stop=(w == nw - 1))
                    if l == 2:
                        pool_s = sm.tile([G, 16], f32, tag="pool_s")
                        nc.vector.tensor_copy(out=pool_s[:, :], in_=pool_p[:, :])
                        nc.sync.dma_start(out=pool_in[:, br * 16:(br + 1) * 16],
                                          in_=pool_s[:, :])
                        pp_ctx.__exit__(None, None, None)
                    pt_ctx.__exit__(None, None, None)
                    pg_ctx.__exit__(None, None, None)

                    if l < 2:
                        nc.gpsimd.collective_compute(
                            "AllGather", mybir.AluOpType.bypass,
                            replica_groups=[list(range(NCORE))],
                            ins=[agin[(br, l)][:, :]], outs=[agfull[(br, l)][:, :]],
                        )
                        half = npad // 2
                        nc.sync.dma_start(out=tabfull[br][0:half, 0:DIMS[l + 1]],
                                          in_=agfull[(br, l)][0:half, :])
                        nc.sync.dma_start(out=tabfull[br][half:npad, 0:DIMS[l + 1]],
                                          in_=agfull[(br, l)][half:npad, :])
            # ---- combined pool AllReduce + tail MLP ----
            nc.gpsimd.collective_compute(
                "AllReduce", mybir.AluOpType.add,
                replica_groups=[list(range(NCORE))],
                ins=[pool_in[:, :]], outs=[pool_out[:, :]],
            )
            pm_ctx = tc.tile_pool(name="pm", bufs=1, space="PSUM")
            pm = pm_ctx.__enter__()
            pool_r = sm.tile([G, 32], f32, tag="pool_r")
            nc.sync.dma_start(out=pool_r[:, :], in_=pool_out[:, :])
            for br in (0, 1):
                pmean = sm.tile([G, 16], f32, tag="pmean")
                nc.vector.tensor_tensor(
                    out=pmean[:, :], in0=pool_r[:, br * 16:(br + 1) * 16],
                    in1=ic_t[br][:, 0:1].to_broadcast([G, 16]),
                    op=mybir.AluOpType.mult)
                pmT_p = pm.tile([16, G], f32, tag="pmT", name=f"pmT_{br}", space="PSUM")
                nc.tensor.transpose(out=pmT_p[:, :], in_=pmean[:, :],
                                    identity=ident[:G, :G])
                pmT_s = sm.tile([16, G], f32, tag="pmTs")
                nc.vector.tensor_copy(out=pmT_s[:, :], in_=pmT_p[:, :])
                p8_p = pm.tile([G, 8], f32, tag="p8", name=f"p8_{br}", space="PSUM")
                nc.tensor.matmul(out=p8_p[:, :], lhsT=pmT_s[:, :], rhs=W3[:, :],
                                 start=True, stop=True)
                nc.vector.tensor_tensor(
                    out=pooled_cat[:, br * 8:(br + 1) * 8], in0=p8_p[:, :],
                    in1=b3[:, :], op=mybir.AluOpType.add)

            pcT_p = pm.tile([16, G], f32, tag="pcT", name="pcT_p", space="PSUM")
            nc.tensor.transpose(out=pcT_p[:, :], in_=pooled_cat[:, :],
                                identity=ident[:G, :G])
            pcT_s = sm.tile([16, G], f32, tag="pcT_s")
            nc.vector.tensor_copy(out=pcT_s[:, :], in_=pcT_p[:, :])
            m1_p = pm.tile([G, 8], f32, tag="m1p", name="m1_p", space="PSUM")
            nc.tensor.matmul(out=m1_p[:, :], lhsT=pcT_s[:, :], rhs=mW1[:, :],
                             start=True, stop=True)
            m1_s = sm.tile([G, 8], f32, tag="m1s")
            nc.vector.tensor_tensor(out=m1_s[:, :], in0=m1_p[:, :], in1=mb1[:, :],
                                    op=mybir.AluOpType.add)
            nc.vector.tensor_tensor(out=m1_s[:, :], in0=m1_s[:, :],
                                    in1=zcol[:G, 0:1].to_broadcast([G, 8]),
                                    op=mybir.AluOpType.max)
            m1T_p = pm.tile([8, G], f32, tag="m1T", name="m1T_p", space="PSUM")
            nc.tensor.transpose(out=m1T_p[:, :], in_=m1_s[:, :], identity=ident[:G, :G])
            m1T_s = sm.tile([8, G], f32, tag="m1Ts")
            nc.vector.tensor_copy(out=m1T_s[:, :], in_=m1T_p[:, :])
            m2_p = pm.tile([G, 2], f32, tag="m2p", name="m2_p", space="PSUM")
            nc.tensor.matmul(out=m2_p[:, :], lhsT=m1T_s[:, :], rhs=mW2[:, :],
                             start=True, stop=True)
            m2_s = sm.tile([G, 2], f32, tag="m2s")
            nc.vector.tensor_tensor(out=m2_s[:, :], in0=m2_p[:, :], in1=mb2[:, :],
                                    op=mybir.AluOpType.add)
            nc.sync.dma_start(out=out_p[:, :], in_=m2_s[:, :])
            pm_ctx.__exit__(None, None, None)

    nc.compile()
    return nc


def _run(inputs, trace=False):
    global last_results
    x0 = np.asarray(inputs["x0"], np.float32)
    x1 = np.asarray(inputs["x1"], np.float32)
    n, d_feat = x0.shape
    ei0 = np.asarray(inputs["edge_index0"])
    ei1 = np.asarray(inputs["edge_index1"])
    b0 = np.asarray(inputs["batch0"])
    b1 = np.asarray(inputs["batch1"])

    per_core = (n + NCORE - 1) // NCORE
    sh = _ceil_to(per_core, P * WG)       # windows per core divisible by WG
    npad = sh * NCORE
    nw = sh // P
    ng = nw // WG
    nchunk = (npad + CHUNK_ROWS - 1) // CHUNK_ROWS
    assert nchunk * CHUNK_ROWS == npad

    pb0 = _prep_branch(ei0, b0, n, npad, sh, nw, ng, nchunk)
    pb1 = _prep_branch(ei1, b1, n, npad, sh, nw, ng, nchunk)
    cap = max(pb0["cap"], pb1["cap"])
    if pb0["cap"] != cap:
        pb0 = _prep_branch(ei0, b0, n, npad, sh, nw, ng, nchunk, cap=cap)
    if pb1["cap"] != cap:
        pb1 = _prep_branch(ei1, b1, n, npad, sh, nw, ng, nchunk, cap=cap)
    bpb = cap // P

    # host-built layer-0 gather tables: dis*x padded to [npad, 128] bf16
    tabs = []
    for x, pb in ((x0, pb0), (x1, pb1)):
        t = np.zeros((npad, P), np.float32)
        t[:n, :d_feat] = x * pb["dis_full"][:n, None]
        tabs.append(t.astype(ml_dtypes.bfloat16))

    ident = np.eye(P, dtype=np.float32)
    iota128 = np.broadcast_to(np.arange(P, dtype=np.float32), (P, P)).astype(ml_dtypes.bfloat16)

    def wgt(name):
        return np.asarray(inputs[name], np.float32)

    common = dict(
        ident=ident, identb=ident.astype(ml_dtypes.bfloat16),
        iota128=np.ascontiguousarray(iota128),
        tab0=tabs[0], tab1=tabs[1],
        W1=wgt("W1").astype(ml_dtypes.bfloat16),
        W2=wgt("W2").astype(ml_dtypes.bfloat16),
        W3=wgt("W3"),
        b1r=np.broadcast_to(wgt("b1"), (P, 32)).copy(),
        b2r=np.broadcast_to(wgt("b2"), (P, 16)).copy(),
        b3r=np.broadcast_to(wgt("b3"), (G, 8)).copy(),
        mW1=wgt("mW1"), mb1r=np.broadcast_to(wgt("mb1"), (G, 8)).copy(),
        mW2=wgt("mW2"), mb2r=np.broadcast_to(wgt("mb2"), (G, 2)).copy(),
        ic0=pb0["inv_cnt"], ic1=pb1["inv_cnt"],
    )
    in_maps = []
    for c in range(NCORE):
        m = dict(common)
        for name, pb in (("0", pb0), ("1", pb1)):
            m[f"idx{name}"] = pb["idx"][c]
            m[f"dl{name}"] = pb["dl"][c]
            m[f"dis{name}"] = pb["dis"][c]
            m[f"oh{name}"] = pb["oh"][c].reshape(P, nw * G)
        in_maps.append(m)

    nc = _build_program(npad, sh, nw, ng, nchunk, bpb, d_feat)
    res = run_bass_kernel_spmd(nc, in_maps, list(range(NCORE)), trace=trace)
    last_results = res
    return np.asarray(res.results[0]["out"], np.float32)


def kernel(**inputs):
    return _run(inputs, trace=False)
